# revision 1
# baseline (speedup 1.0000x reference)
"""BiMambaBlock Trainium2 kernel (8-core SPMD via Bass/Tile).

Sharding: core = (b, dir, s) with b in {0,1} batch, dir in {fwd, bwd},
s in {0,1} half of d_inner (2048 -> 1024 per core).

Launch 1 (per core): layernorm (folded into W_in) -> in_proj -> causal
depthwise conv + silu -> x_proj -> dt_proj/softplus -> selective scan
(tensor_tensor_scan over time per (state n, 128-channel block)) ->
gate by silu(z) -> out_proj partial (@ W_out rows of the Di half).
bwd direction runs on host-reversed time.

Launch 2 (per core = (b, quarter of L)): residual + bias + partial sums,
then comb=[fwd,bwd] @ Wg/Wv gated combine.

Everything operates in time-transposed layout [feature_partitions, L].
"""

import os
import sys

sys.path.insert(0, "/opt/trn_rl_repo")

# CoreSim has no Silu table; when set, lower silu to sigmoid+mul (HW uses
# the fused Silu activation).
SIM_SAFE = bool(os.environ.get("KERNEL_SIM_SAFE"))

import numpy as np
import ml_dtypes

import concourse.bass as bass
import concourse.mybir as mybir
import concourse.tile as tile
from concourse import bacc
from concourse.bass_utils import run_bass_kernel_spmd

FP32 = mybir.dt.float32
BF16 = mybir.dt.bfloat16
AF = mybir.ActivationFunctionType
OP = mybir.AluOpType
BF = ml_dtypes.bfloat16

B, L, Dm, Di, N, R, KC = 2, 1024, 1024, 2048, 16, 64, 4
DiS = Di // 2  # 1024 channels per core
EPS = 1e-5
NCORES = 8


# ----------------------------------------------------------------- launch 1
def build_launch1():
    nc = bacc.Bacc("TRN2", target_bir_lowering=False, debug=False,
                   num_devices=NCORES)
    xT = nc.dram_tensor("xT", [Dm, L], FP32, kind="ExternalInput")
    w_in = nc.dram_tensor("w_in", [Dm, 3072], BF16, kind="ExternalInput")
    w_in_c = nc.dram_tensor("w_in_c", [1, 3072], BF16, kind="ExternalInput")
    b_in = nc.dram_tensor("b_in", [3072, 1], FP32, kind="ExternalInput")
    conv_w = nc.dram_tensor("conv_w", [Di, KC], FP32, kind="ExternalInput")
    conv_b = nc.dram_tensor("conv_b", [Di, 1], FP32, kind="ExternalInput")
    w_xp = nc.dram_tensor("w_xp", [Di, 96], BF16, kind="ExternalInput")
    w_dt = nc.dram_tensor("w_dt", [R, DiS], BF16, kind="ExternalInput")
    b_dt = nc.dram_tensor("b_dt", [DiS, 1], FP32, kind="ExternalInput")
    a_mat = nc.dram_tensor("a_mat", [DiS, N], FP32, kind="ExternalInput")
    d_vec = nc.dram_tensor("d_vec", [DiS, 1], FP32, kind="ExternalInput")
    w_out = nc.dram_tensor("w_out", [DiS, Dm], BF16, kind="ExternalInput")
    eye = nc.dram_tensor("eye", [2 * N, 2 * N * 128], BF16, kind="ExternalInput")
    ident = nc.dram_tensor("ident", [128, 128], BF16, kind="ExternalInput")
    p_out = nc.dram_tensor("p_out", [Dm, L], FP32, kind="ExternalOutput")

    NXP = Di // 128        # 16 xp channel tiles
    NSH = DiS // 128       # 8 shard channel tiles
    NMD = Dm // 128        # 8 model-dim tiles

    with tile.TileContext(nc) as tc:
        with (
            tc.tile_pool(name="pers", bufs=1) as pers,
            tc.tile_pool(name="bias", bufs=2) as biasp,
        ):
            # --- persistent tiles (whole-kernel lifetime) ---
            silu_z = [pers.tile([128, L], BF16, name=f"sz{i}", tag=f"sz{i}")
                      for i in range(NSH)]
            xp_bf = [pers.tile([128, L], BF16, name=f"xp{i}", tag=f"xp{i}")
                     for i in range(NXP)]
            deltaT = [pers.tile([128, L], FP32, name=f"dl{i}", tag=f"dl{i}")
                      for i in range(NSH)]
            w_t = [pers.tile([128, L], BF16, name=f"wt{i}", tag=f"wt{i}")
                   for i in range(NSH)]
            y_acc = [pers.tile([128, L], BF16, name=f"ya{i}", tag=f"ya{i}")
                     for i in range(NSH)]
            a_sb = [pers.tile([128, N], FP32, name=f"a{i}", tag=f"a{i}")
                    for i in range(NSH)]
            d_sb = [pers.tile([128, 1], FP32, name=f"d{i}", tag=f"d{i}")
                    for i in range(NSH)]
            ident_sb = pers.tile([128, 128], BF16, name="identsb",
                                 tag="identsb")
            nc.sync.dma_start(ident_sb[:], ident.ap())
            ones_f = pers.tile([1, 128], FP32, name="onesf", tag="onesf")
            ones_r = pers.tile([128, 1], FP32, name="onesr", tag="onesr")
            nc.vector.memset(ones_f[:], 1.0)
            nc.vector.memset(ones_r[:], 1.0)
            for i in range(NSH):
                nc.sync.dma_start(a_sb[i][:], a_mat.ap()[i * 128:(i + 1) * 128, :])
                nc.sync.dma_start(d_sb[i][:], d_vec.ap()[i * 128:(i + 1) * 128, :])

            # ============ phase IP: LN stats + z1 + in_proj ============
            with (
                tc.tile_pool(name="ip", bufs=1) as ip,
                tc.tile_pool(name="wks", bufs=3) as wks,
                tc.tile_pool(name="cvt", bufs=2) as cvt,
            ):
                z1 = [ip.tile([128, L], BF16, name=f"z1{i}", tag=f"z1{i}")
                      for i in range(NMD)]
                mu = ip.tile([1, L], FP32, name="mu", tag="mu")
                rstd = ip.tile([1, L], FP32, name="rstd", tag="rstd")
                mr_row = ip.tile([1, L], BF16, name="mr", tag="mr")
                rstd_b = ip.tile([128, L], FP32, name="rstdb", tag="rstdb")
                wc = ip.tile([1, 3072], BF16, name="wc", tag="wc")
                nc.sync.dma_start(wc[:], w_in_c.ap())

                # ---- pass 1: stats ----
                with (
                    tc.tile_pool(name="sta", bufs=2) as sta,
                    tc.tile_pool(name="psst", bufs=1, space="PSUM") as psst,
                ):
                    ps_mu = psst.tile([1, L], FP32, name="psmu", tag="psmu")
                    ps_sq = psst.tile([1, L], FP32, name="pssq", tag="pssq")
                    for i in range(NMD):
                        xti = sta.tile([128, L], FP32, name="xti", tag="xti")
                        nc.sync.dma_start(xti[:], xT.ap()[i * 128:(i + 1) * 128, :])
                        x2i = sta.tile([128, L], FP32, name="x2i", tag="x2i", bufs=1)
                        nc.scalar.activation(x2i[:], xti[:], AF.Square)
                        for h in range(2):
                            sl = slice(h * 512, (h + 1) * 512)
                            nc.tensor.matmul(ps_mu[:, sl], ones_r[:], xti[:, sl],
                                             start=(i == 0), stop=(i == NMD - 1))
                            nc.tensor.matmul(ps_sq[:, sl], ones_r[:], x2i[:, sl],
                                             start=(i == 0), stop=(i == NMD - 1))
                    nc.scalar.mul(mu[:], ps_mu[:], 1.0 / Dm)
                    msq = sta.tile([1, L], FP32, name="strow", tag="strow", bufs=3)
                    nc.scalar.mul(msq[:], ps_sq[:], 1.0 / Dm)
                    mu2 = sta.tile([1, L], FP32, name="strow", tag="strow", bufs=3)
                    nc.vector.tensor_tensor(mu2[:], mu[:], mu[:], OP.mult)
                    var = sta.tile([1, L], FP32, name="strow", tag="strow", bufs=3)
                    nc.vector.tensor_tensor(var[:], msq[:], mu2[:], OP.subtract)
                    sd = sta.tile([1, L], FP32, name="strow", tag="strow", bufs=3)
                    eps_t = sta.tile([1, 1], FP32, name="epst", tag="epst")
                    nc.vector.memset(eps_t[:], EPS)
                    nc.scalar.activation(sd[:], var[:], AF.Sqrt, bias=eps_t[:])
                    nc.vector.reciprocal(rstd[:], sd[:])
                    nc.vector.tensor_tensor(mr_row[:], mu[:], rstd[:], OP.mult)
                    for h in range(2):
                        psb = psst.tile([128, 512], FP32, name="psb0", tag="psb0")
                        nc.tensor.matmul(psb[:], ones_f[:],
                                         rstd[:, h * 512:(h + 1) * 512],
                                         start=True, stop=True)
                        nc.scalar.copy(rstd_b[:, h * 512:(h + 1) * 512], psb[:])
                    # ---- pass 2: z1 = xT * rstd (reload xT) ----
                    for i in range(NMD):
                        xti = sta.tile([128, L], FP32, name="xti", tag="xti")
                        nc.sync.dma_start(xti[:], xT.ap()[i * 128:(i + 1) * 128, :])
                        nc.vector.tensor_tensor(z1[i][:], xti[:], rstd_b[:],
                                                OP.mult)

                # ---- in_proj: groups of 4 output tiles, stream weights ----
                with tc.tile_pool(name="psin", bufs=6, space="PSUM") as psin:
                    for mg in range(8):           # 8 groups x 3 mtiles
                        pst = [psin.tile([128, 512], FP32, name="psi",
                                         tag="psi") for _ in range(6)]
                        for kt in range(NMD):
                            wkt = wks.tile([128, 384], BF16, name="wkt",
                                           tag="wkt")
                            nc.sync.dma_start(
                                wkt[:],
                                w_in.ap()[kt * 128:(kt + 1) * 128,
                                          mg * 384:(mg + 1) * 384])
                            for m4 in range(3):
                                for h in range(2):
                                    nc.tensor.matmul(
                                        pst[m4 * 2 + h][:],
                                        wkt[:, m4 * 128:(m4 + 1) * 128],
                                        z1[kt][:, h * 512:(h + 1) * 512],
                                        start=(kt == 0), stop=False)
                        for m4 in range(3):
                            mt = mg * 3 + m4
                            for h in range(2):
                                nc.tensor.matmul(
                                    pst[m4 * 2 + h][:],
                                    wc[:, mt * 128:(mt + 1) * 128],
                                    mr_row[:, h * 512:(h + 1) * 512],
                                    start=False, stop=True)
                        for m4 in range(3):
                            mt = mg * 3 + m4
                            bi = biasp.tile([128, 1], FP32, name="bin",
                                            tag="bin")
                            nc.sync.dma_start(
                                bi[:], b_in.ap()[mt * 128:(mt + 1) * 128, :])
                            if mt < NXP:
                                # evict to padded bf16 tile, then causal
                                # conv as 4 psum-accumulated diag matmuls
                                xpad = cvt.tile([128, L + 3], BF16,
                                                name="xpad", tag="xpad")
                                nc.vector.memset(xpad[:, 0:3], 0.0)
                                for h in range(2):
                                    nc.scalar.activation(
                                        xpad[:, 3 + h * 512:3 + (h + 1) * 512],
                                        pst[m4 * 2 + h][:], AF.Identity,
                                        bias=bi[:])
                                cw = biasp.tile([128, KC], FP32, name="cw",
                                                tag="cw")
                                cb = biasp.tile([128, 1], FP32, name="cb",
                                                tag="cb")
                                nc.sync.dma_start(
                                    cw[:], conv_w.ap()[mt * 128:(mt + 1) * 128, :])
                                nc.sync.dma_start(
                                    cb[:], conv_b.ap()[mt * 128:(mt + 1) * 128, :])
                                diags = []
                                for j in range(KC):
                                    dg = cvt.tile([128, 128], BF16,
                                                  name="diag", tag="diag",
                                                  bufs=8)
                                    nc.vector.tensor_scalar_mul(
                                        dg[:], ident_sb[:], cw[:, j:j + 1])
                                    diags.append(dg)
                                for h in range(2):
                                    pcv = psin.tile([128, 512], FP32,
                                                    name="pcv", tag="pcv",
                                                    bufs=2)
                                    for j in range(KC):
                                        nc.tensor.matmul(
                                            pcv[:], diags[j][:],
                                            xpad[:, j + h * 512:
                                                 j + h * 512 + 512],
                                            start=(j == 0), stop=(j == KC - 1))
                                    cs = slice(h * 512, (h + 1) * 512)
                                    if SIM_SAFE:
                                        sg = cvt.tile([128, 512], BF16,
                                                      name="sg", tag="sg")
                                        nc.scalar.activation(
                                            sg[:], pcv[:], AF.Sigmoid,
                                            bias=cb[:])
                                        zz = cvt.tile([128, 512], BF16,
                                                      name="zz", tag="zz")
                                        nc.scalar.activation(
                                            zz[:], pcv[:], AF.Identity,
                                            bias=cb[:])
                                        nc.vector.tensor_tensor(
                                            xp_bf[mt][:, cs], zz[:], sg[:],
                                            OP.mult)
                                    else:
                                        nc.scalar.activation(
                                            xp_bf[mt][:, cs], pcv[:],
                                            AF.Silu, bias=cb[:])
                            else:
                                zt = silu_z[mt - NXP]
                                for h in range(2):
                                    cs = slice(h * 512, (h + 1) * 512)
                                    if SIM_SAFE:
                                        sg = cvt.tile([128, 512], BF16,
                                                      name="sg2", tag="sg2")
                                        nc.scalar.activation(
                                            sg[:], pst[m4 * 2 + h][:],
                                            AF.Sigmoid, bias=bi[:])
                                        zz = cvt.tile([128, 512], BF16,
                                                      name="zz2", tag="zz2")
                                        nc.scalar.activation(
                                            zz[:], pst[m4 * 2 + h][:],
                                            AF.Identity, bias=bi[:])
                                        nc.vector.tensor_tensor(
                                            zt[:, cs], zz[:], sg[:], OP.mult)
                                    else:
                                        nc.scalar.activation(
                                            zt[:, cs],
                                            pst[m4 * 2 + h][:], AF.Silu,
                                            bias=bi[:])

            # ============ phase XD: x_proj, dt_proj, w, y-init ============
            with tc.tile_pool(name="mid", bufs=1) as midp:
              dbl_bf = midp.tile([96, L], BF16, name="dbl", tag="dbl")
              bc_pack = midp.tile([2 * N, L], BF16, name="bcp", tag="bcp")
              eye_sb = midp.tile([2 * N, 2 * N * 128], BF16, name="eyesb",
                                 tag="eyesb")
              nc.sync.dma_start(eye_sb[:], eye.ap())

              with (
                tc.tile_pool(name="xd", bufs=1) as xd,
                tc.tile_pool(name="psxd", bufs=2, space="PSUM") as psxd,
                tc.tile_pool(name="dbf", bufs=2) as dbfp,
            ):
                wx = [xd.tile([128, 96], BF16, name=f"wx{i}", tag=f"wx{i}")
                      for i in range(NXP)]
                for i in range(NXP):
                    nc.sync.dma_start(wx[i][:], w_xp.ap()[i * 128:(i + 1) * 128, :])
                for h in range(2):
                    psd = psxd.tile([96, 512], FP32, name="psd", tag="psd")
                    for kt in range(NXP):
                        nc.tensor.matmul(psd[:], wx[kt][:],
                                         xp_bf[kt][:, h * 512:(h + 1) * 512],
                                         start=(kt == 0), stop=(kt == NXP - 1))
                    nc.scalar.copy(dbl_bf[:, h * 512:(h + 1) * 512], psd[:])
                # move B/C rows to a partition-0 tile (PE base match)
                nc.sync.dma_start(bc_pack[:], dbl_bf[R:R + 2 * N, :])

                wdt = xd.tile([R, DiS], BF16, name="wdt", tag="wdt")
                nc.sync.dma_start(wdt[:], w_dt.ap())
                for mt in range(NSH):
                    bdt = biasp.tile([128, 1], FP32, name="bdt", tag="bdt")
                    nc.sync.dma_start(bdt[:], b_dt.ap()[mt * 128:(mt + 1) * 128, :])
                    for h in range(2):
                        psdt = psxd.tile([128, 512], FP32, name="psdt",
                                         tag="psdt")
                        nc.tensor.matmul(psdt[:],
                                         wdt[:, mt * 128:(mt + 1) * 128],
                                         dbl_bf[0:R, h * 512:(h + 1) * 512],
                                         start=True, stop=True)
                        # softplus(x) = ln(1 + exp(x)); x = psum + b_dt is
                        # always << 80 here so exp cannot overflow
                        edt = dbfp.tile([128, 512], FP32, name="edt",
                                        tag="edt")
                        nc.scalar.activation(edt[:], psdt[:], AF.Exp,
                                             bias=bdt[:])
                        nc.scalar.activation(
                            deltaT[mt][:, h * 512:(h + 1) * 512],
                            edt[:], AF.Ln, bias=ones_r[:, 0:1])
                for mt in range(NSH):
                    dbf = dbfp.tile([128, L], BF16, name="dbft", tag="dbft")
                    nc.scalar.copy(dbf[:], deltaT[mt][:])
                    nc.vector.tensor_tensor(w_t[mt][:], dbf[:], xp_bf[mt][:],
                                            OP.mult)
                    nc.vector.tensor_scalar_mul(y_acc[mt][:], xp_bf[mt][:],
                                                d_sb[mt][:])

              # ============ phase SC: selective scan over n ============
              # dBx on GPSIMD, scans + prod on DVE, y accumulated on PE via
              # identity-matmul into fp32 PSUM (3 passes of <=3 mtiles to fit
              # 6 psum banks + 2 bcast banks)
              with (
                  tc.tile_pool(name="rep", bufs=3) as repp,
                  tc.tile_pool(name="scan", bufs=4) as scanp,
                  tc.tile_pool(name="psr", bufs=2, space="PSUM") as psr,
                  tc.tile_pool(name="psy", bufs=1, space="PSUM") as psy,
              ):
                  for mts in ((0, 1, 2), (3, 4, 5), (6, 7)):
                      yps = {mt: psy.tile([128, L], FP32, name=f"psy{j}",
                                          tag=f"psy{j}")
                             for j, mt in enumerate(mts)}
                      for mt in mts:
                          for h in range(2):
                              nc.tensor.matmul(
                                  yps[mt][:, h * 512:(h + 1) * 512],
                                  ident_sb[:],
                                  y_acc[mt][:, h * 512:(h + 1) * 512],
                                  start=True, stop=False)
                      for n in range(N):
                          b_rep = repp.tile([128, L], BF16, name="brep",
                                            tag="brep")
                          c_rep = repp.tile([128, L], BF16, name="crep",
                                            tag="crep")
                          for (row, rep) in ((n, b_rep), (N + n, c_rep)):
                              for h in range(2):
                                  psb = psr.tile([128, 512], FP32,
                                                 name="psrep", tag="psrep")
                                  nc.tensor.matmul(
                                      psb[:],
                                      eye_sb[:, row * 128:(row + 1) * 128],
                                      bc_pack[:, h * 512:(h + 1) * 512],
                                      start=True, stop=True)
                                  nc.scalar.copy(
                                      rep[:, h * 512:(h + 1) * 512], psb[:])
                          for mt in mts:
                              dA = scanp.tile([128, L], BF16, name="dA",
                                              tag="dA")
                              nc.scalar.activation(
                                  dA[:], deltaT[mt][:], AF.Exp,
                                  scale=a_sb[mt][:, n:n + 1])
                              dBx = scanp.tile([128, L], BF16, name="dBx",
                                               tag="dBx")
                              nc.vector.tensor_tensor(dBx[:], w_t[mt][:],
                                                      b_rep[:], OP.mult)
                              hh = scanp.tile([128, L], BF16, name="hh",
                                              tag="hh")
                              nc.vector.tensor_tensor_scan(
                                  hh[:], dA[:], dBx[:], 0.0, OP.mult, OP.add)
                              prod = scanp.tile([128, L], BF16, name="prod",
                                                tag="prod")
                              nc.vector.tensor_tensor(prod[:], hh[:],
                                                      c_rep[:], OP.mult)
                              for h in range(2):
                                  nc.tensor.matmul(
                                      yps[mt][:, h * 512:(h + 1) * 512],
                                      ident_sb[:],
                                      prod[:, h * 512:(h + 1) * 512],
                                      start=False, stop=(n == N - 1))
                      for mt in mts:
                          nc.scalar.copy(y_acc[mt][:], yps[mt][:])

            # ============ phase FN: y*silu(z), out_proj ============
            with (
                tc.tile_pool(name="fin", bufs=1) as finp,
                tc.tile_pool(name="pso", bufs=4, space="PSUM") as pso_p,
                tc.tile_pool(name="ob", bufs=2) as obp,
            ):
                wo_sb = [finp.tile([128, Dm], BF16, name=f"wo{i}", tag=f"wo{i}")
                         for i in range(NSH)]
                for i in range(NSH):
                    nc.sync.dma_start(wo_sb[i][:],
                                      w_out.ap()[i * 128:(i + 1) * 128, :])
                yf = [finp.tile([128, L], BF16, name=f"yf{i}", tag=f"yf{i}")
                      for i in range(NSH)]
                for mt in range(NSH):
                    nc.vector.tensor_tensor(yf[mt][:], y_acc[mt][:],
                                            silu_z[mt][:], OP.mult)
                for mo in range(NMD):
                    osb = obp.tile([128, L], FP32, name="osb", tag="osb")
                    for h in range(2):
                        pso = pso_p.tile([128, 512], FP32, name="pso",
                                         tag="pso")
                        for kt in range(NSH):
                            nc.tensor.matmul(
                                pso[:], wo_sb[kt][:, mo * 128:(mo + 1) * 128],
                                yf[kt][:, h * 512:(h + 1) * 512],
                                start=(kt == 0), stop=(kt == NSH - 1))
                        nc.scalar.copy(osb[:, h * 512:(h + 1) * 512], pso[:])
                    nc.sync.dma_start(p_out.ap()[mo * 128:(mo + 1) * 128, :],
                                      osb[:])

    nc.compile()
    return nc


# ----------------------------------------------------------------- launch 2
LQ = L // 4  # 256 rows per core


def build_launch2():
    nc = bacc.Bacc("TRN2", target_bir_lowering=False, debug=False,
                   num_devices=NCORES)
    xbt = nc.dram_tensor("xbt", [Dm, LQ], BF16, kind="ExternalInput")
    pf0 = nc.dram_tensor("pf0", [Dm, LQ], BF16, kind="ExternalInput")
    pf1 = nc.dram_tensor("pf1", [Dm, LQ], BF16, kind="ExternalInput")
    pb0 = nc.dram_tensor("pb0", [Dm, LQ], BF16, kind="ExternalInput")
    pb1 = nc.dram_tensor("pb1", [Dm, LQ], BF16, kind="ExternalInput")
    wg = nc.dram_tensor("wg", [2 * Dm, Dm], BF16, kind="ExternalInput")
    wv = nc.dram_tensor("wv", [2 * Dm, Dm], BF16, kind="ExternalInput")
    b_of = nc.dram_tensor("b_of", [Dm, 1], FP32, kind="ExternalInput")
    b_ob = nc.dram_tensor("b_ob", [Dm, 1], FP32, kind="ExternalInput")
    bg = nc.dram_tensor("bg", [Dm, 1], FP32, kind="ExternalInput")
    bv = nc.dram_tensor("bv", [Dm, 1], FP32, kind="ExternalInput")
    ot = nc.dram_tensor("ot", [Dm, LQ], FP32, kind="ExternalOutput")

    NMD = Dm // 128

    with tile.TileContext(nc) as tc:
        with (
            tc.tile_pool(name="pers", bufs=1) as pers,
            tc.tile_pool(name="tx", bufs=3) as txp,
            tc.tile_pool(name="ps", bufs=2, space="PSUM") as psp,
        ):
            wgt = [pers.tile([128, Dm], BF16, name=f"wg{i}", tag=f"wg{i}") for i in range(16)]
            wvt = [pers.tile([128, Dm], BF16, name=f"wv{i}", tag=f"wv{i}") for i in range(16)]
            for i in range(16):
                nc.sync.dma_start(wgt[i][:], wg.ap()[i * 128:(i + 1) * 128, :])
                nc.sync.dma_start(wvt[i][:], wv.ap()[i * 128:(i + 1) * 128, :])
            fwd_bf = [pers.tile([128, LQ], BF16, name=f"fb{i}", tag=f"fb{i}") for i in range(NMD)]
            bwd_bf = [pers.tile([128, LQ], BF16, name=f"bb{i}", tag=f"bb{i}") for i in range(NMD)]
            s_sum = [pers.tile([128, LQ], FP32, name=f"s{i}", tag=f"s{i}") for i in range(NMD)]
            for i in range(NMD):
                sl = slice(i * 128, (i + 1) * 128)
                tx = txp.tile([128, LQ], BF16, name="tx", tag="tx")
                ta = txp.tile([128, LQ], BF16, name="ta", tag="ta")
                tb2 = txp.tile([128, LQ], BF16, name="tb2", tag="tb2")
                bo_f = txp.tile([128, 1], FP32, name="bof", tag="bof")
                bo_b = txp.tile([128, 1], FP32, name="bob", tag="bob")
                nc.sync.dma_start(bo_f[:], b_of.ap()[sl, :])
                nc.sync.dma_start(bo_b[:], b_ob.ap()[sl, :])
                nc.sync.dma_start(tx[:], xbt.ap()[sl, :])
                p0 = txp.tile([128, LQ], BF16, name="p0", tag="p0")
                p1 = txp.tile([128, LQ], BF16, name="p1", tag="p1")
                nc.sync.dma_start(p0[:], pf0.ap()[sl, :])
                nc.sync.dma_start(p1[:], pf1.ap()[sl, :])
                nc.vector.tensor_tensor(ta[:], p0[:], p1[:], OP.add)
                nc.vector.scalar_tensor_tensor(
                    fwd_bf[i][:], tx[:], bo_f[:], ta[:], OP.add, OP.add)
                p2 = txp.tile([128, LQ], BF16, name="p0", tag="p0")
                p3 = txp.tile([128, LQ], BF16, name="p1", tag="p1")
                nc.sync.dma_start(p2[:], pb0.ap()[sl, :])
                nc.sync.dma_start(p3[:], pb1.ap()[sl, :])
                nc.vector.tensor_tensor(tb2[:], p2[:], p3[:], OP.add)
                nc.vector.scalar_tensor_tensor(
                    bwd_bf[i][:], tx[:], bo_b[:], tb2[:], OP.add, OP.add)
                nc.vector.tensor_tensor(s_sum[i][:], fwd_bf[i][:], bwd_bf[i][:],
                                        OP.add)

            comb = fwd_bf + bwd_bf
            for mo in range(NMD):
                bgt = txp.tile([128, 1], FP32, name="bgt", tag="bgt")
                bvt = txp.tile([128, 1], FP32, name="bvt", tag="bvt")
                sl = slice(mo * 128, (mo + 1) * 128)
                nc.sync.dma_start(bgt[:], bg.ap()[sl, :])
                nc.sync.dma_start(bvt[:], bv.ap()[sl, :])
                psg = psp.tile([128, LQ], FP32, name="psg", tag="psg")
                psv = psp.tile([128, LQ], FP32, name="psv", tag="psv")
                for kt in range(16):
                    nc.tensor.matmul(psg[:], wgt[kt][:, sl], comb[kt][:],
                                     start=(kt == 0), stop=(kt == 15))
                for kt in range(16):
                    nc.tensor.matmul(psv[:], wvt[kt][:, sl], comb[kt][:],
                                     start=(kt == 0), stop=(kt == 15))
                g = txp.tile([128, LQ], FP32, name="g", tag="g")
                v = txp.tile([128, LQ], FP32, name="v", tag="v")
                nc.scalar.activation(g[:], psg[:], AF.Sigmoid, bias=bgt[:])
                nc.scalar.activation(v[:], psv[:], AF.Identity, bias=bvt[:])
                d = txp.tile([128, LQ], FP32, name="dd", tag="dd")
                nc.vector.tensor_tensor(d[:], v[:], s_sum[mo][:], OP.subtract)
                m = txp.tile([128, LQ], FP32, name="mm", tag="mm")
                nc.vector.tensor_tensor(m[:], g[:], d[:], OP.mult)
                o = txp.tile([128, LQ], FP32, name="oo", tag="oo")
                nc.vector.tensor_tensor(o[:], m[:], s_sum[mo][:], OP.add)
                o2 = txp.tile([128, LQ], FP32, name="o2", tag="o2")
                nc.scalar.mul(o2[:], o[:], 0.5)
                nc.sync.dma_start(ot.ap()[sl, :], o2[:])

    nc.compile()
    return nc


# ------------------------------------------------------------------- host
_cache = {}


def _get_nc(which):
    if which not in _cache:
        _cache[which] = build_launch1() if which == 1 else build_launch2()
    return _cache[which]


def prep_launch1_inmaps(x, ln_w, ln_b, W_in, b_in, conv_w, conv_b, W_xproj,
                        W_dt, b_dt, A_log, D, W_out, b_out):
    """Build the 8 per-core input dicts for launch 1."""
    in_maps = []
    eye = np.zeros((2 * N, 2 * N * 128), np.float32)
    for j in range(2 * N):
        eye[j, j * 128:(j + 1) * 128] = 1.0
    eye = eye.astype(BF)
    ident_np = np.eye(128, dtype=np.float32).astype(BF)
    xf = [np.ascontiguousarray(x[b].T) for b in range(B)]           # [Dm, L]
    xr = [np.ascontiguousarray(x[b, ::-1].T) for b in range(B)]     # reversed
    for core in range(NCORES):
        b, dr, s = core >> 2, (core >> 1) & 1, core & 1
        sl = slice(s * DiS, (s + 1) * DiS)
        W_eff = ln_w[dr][:, None] * W_in[dr]                         # [Dm, 4096]
        b_eff = ln_b[dr] @ W_in[dr] + b_in[dr]                       # [4096]
        # xp channel permutation: the device scans xp tiles 0..7, so this
        # core's Di-shard channels must come first
        perm = np.concatenate([np.arange(s * DiS, (s + 1) * DiS),
                               np.arange((1 - s) * DiS, (2 - s) * DiS)])
        cols = np.concatenate([perm, Di + s * DiS + np.arange(DiS)])
        Wc = W_eff[:, cols]                                          # [Dm, 3072]
        in_maps.append({
            "xT": (xf if dr == 0 else xr)[b],
            "w_in": Wc.astype(BF),
            "w_in_c": (-Wc.sum(0, keepdims=True)).astype(BF),
            "b_in": b_eff[cols][:, None].astype(np.float32),
            "conv_w": conv_w[dr][perm].astype(np.float32),
            "conv_b": conv_b[dr][perm][:, None].astype(np.float32),
            "w_xp": W_xproj[dr][perm].astype(BF),
            "w_dt": W_dt[dr][:, sl].astype(BF),
            "b_dt": b_dt[dr][sl][:, None].astype(np.float32),
            "a_mat": (-np.exp(A_log[dr][sl])).astype(np.float32),
            "d_vec": D[dr][sl][:, None].astype(np.float32),
            "w_out": W_out[dr][sl, :].astype(BF),
            "eye": eye,
            "ident": ident_np,
        })
    return in_maps, xf


def prep_launch2_inmaps(res1, xf, Wg, bg, Wv, bv, b_out):
    """res1: list of 8 result dicts from launch 1."""
    idx = lambda b, dr, s: (b << 2) | (dr << 1) | s
    pf = [[res1[idx(b, 0, s)]["p_out"] for s in range(2)] for b in range(B)]
    pb = [[res1[idx(b, 1, s)]["p_out"][:, ::-1] for s in range(2)]
          for b in range(B)]
    wg_bf = Wg.astype(BF)
    wv_bf = Wv.astype(BF)
    in_maps = []
    for core in range(NCORES):
        b, q = core >> 2, core & 3
        cs = slice(q * LQ, (q + 1) * LQ)
        in_maps.append({
            "xbt": np.ascontiguousarray(xf[b][:, cs]).astype(BF),
            "pf0": np.ascontiguousarray(pf[b][0][:, cs]).astype(BF),
            "pf1": np.ascontiguousarray(pf[b][1][:, cs]).astype(BF),
            "pb0": np.ascontiguousarray(pb[b][0][:, cs]).astype(BF),
            "pb1": np.ascontiguousarray(pb[b][1][:, cs]).astype(BF),
            "wg": wg_bf, "wv": wv_bf,
            "b_of": b_out[0][:, None].astype(np.float32),
            "b_ob": b_out[1][:, None].astype(np.float32),
            "bg": bg[:, None].astype(np.float32),
            "bv": bv[:, None].astype(np.float32),
        })
    return in_maps


def kernel(x, ln_w, ln_b, W_in, b_in, conv_w, conv_b, W_xproj, W_dt, b_dt,
           A_log, D, W_out, b_out, Wg, bg, Wv, bv):
    x = np.asarray(x, np.float32)
    args = [np.asarray(a, np.float32) for a in
            (ln_w, ln_b, W_in, b_in, conv_w, conv_b, W_xproj, W_dt, b_dt,
             A_log, D, W_out, b_out)]
    Wg, bg, Wv, bv = (np.asarray(a, np.float32) for a in (Wg, bg, Wv, bv))

    in1, xf = prep_launch1_inmaps(x, *args)
    nc1 = _get_nc(1)
    res1 = run_bass_kernel_spmd(nc1, in1, core_ids=list(range(NCORES))).results

    in2 = prep_launch2_inmaps(res1, xf, Wg, bg, Wv, bv, args[-1])
    nc2 = _get_nc(2)
    res2 = run_bass_kernel_spmd(nc2, in2, core_ids=list(range(NCORES))).results

    out = np.empty((B, L, Dm), np.float32)
    for core in range(NCORES):
        b, q = core >> 2, core & 3
        out[b, q * LQ:(q + 1) * LQ, :] = res2[core]["ot"].T
    return out

